# revision 99
# baseline (speedup 1.0000x reference)
"""MultiHeadAttention (B=2, S=2048, HID=1024, NH=16, HD=64, RoPE) on 8 TRN2 cores.

Sharding: 8 cores = 2 batches x 4 head-groups (4 heads per core).
Per core: q/k/v projections for its 4 heads (tensor parallel on H), RoPE,
attention, and a partial o-projection over its 256 channels. Host sums the
4 partial o-projections per batch (the TP unshard) and adds bo.

All compute tensors are bf16 (x, weights, RoPE tables, q/k/v, softmax
weights); matmuls accumulate in f32 PSUM. The partial o-projection is
stored bf16 (the host accumulates the four partials in f32), halving the
output DMA.

RoPE without cross-partition ops: the q/k projection weight columns are
split into an L set (channels 0-31 of each head) and an H set (channels
32-63), so each PSUM partition holds a channel and its rotate-half partner
at the same partition index in two PSUM banks. RoPE is then two full-width
DVE muls (the sin term reads the PSUM pair dim reversed; signs live in the
sin table) plus one Pool add -> bf16 [128, 2, S].

Attention: scores per (head, k-tile) are bf16 matmuls ([64,128] x
[64,512]); exp on ACT with scale=1/8, bias=-4 (the bias cancels in the
row-sum normalization). AV runs transposed -- p [128k,128q] stationary,
v [128k,65] moving (65th col = ones accumulates the row sums) -- packing
the 64-wide head dim into the free axis at full 128-partition occupancy.
Normalization is a per-partition tensor_scalar mul with the reciprocal row
sums; DMA transposes put the normalized attention back in [channel, seq]
for the o-projection.

Schedule (the big lever -- ACT exp is ~133us busy and PE ~140us, so the
span is set by how tightly both pipelines pack):
- PE warm-up: ~30 junk matmuls starting at t~0.3us keep the tensor engine
  continuously busy through the initial input DMAs, so the cost model's
  p-state ramp (0.65/1.2 GHz until 3us of continuous execution) is spent
  on throwaway work and k/q projections run at full 2.4 GHz.
- DMA order follows first-use: wk/x0/wq in ko halves, then per-chunk
  cos/sin and x just ahead of that chunk's projection + RoPE.
- First exp ~19us in: emit k0, q0, k1, then steps 0 and 1 interleaved
  group-wise (step 1's exps depend only on the Pool-side hh1 RoPE adds,
  which complete during step 0's DVE rope-chain waits), with k2/k3 and
  the first v tiles woven between groups.
- All other work is piecewise: late q chunks as 4-matmul pieces, drains
  as per-q-tile AV pieces + a finish, o-projections per s-tile -- all
  distributed over each step's 8 group slots with per-step PE load kept
  just under the ~8.3us ACT step time. Per step: o-projections first
  (their avt inputs are a step old), older drains, then lag-1 drains
  (whose AV needs the previous step's last exp).
- Tail after the last exp: per-q-tile pipeline of AV (two alternating
  PSUM banks -- a start=True matmul re-zeroes its whole 2KB bank, which
  would WAR against the previous q-tile's norm), norm, PE transpose via
  an identity matrix (the DMA-transpose queue serializes at 625ns/issue),
  with PSUM->SBUF copies split DVE/ACT and per-oc stores on alternating
  DMA queues.

PSUM (8 banks): [128,2,512] f32 score-pair ring (tag mm, bufs=2 -> 4
banks, also k0/k1 and the tail transposes), one [128,2,512] projection
slot (tag proj, 2 banks) for all later q/k chunks, and a [128,512] ring
(tag bp, bufs=2 -> 2 banks) shared by warm-up, v-proj, AV accumulators,
and o-proj.

Hardware constraints found the hard way: Pool/GPSIMD cannot touch PSUM
(BIR verifier), only one open PSUM accumulation group per 2KB bank, DMA
transposes only on the SP/ACT hardware DGE queues, and the Tile
scheduler orders a consumer only against producers already emitted (so
emission order is part of correctness, not just performance).
"""

import numpy as np

B, S, HID = 2, 2048, 1024
NH, HD = 16, 64
BASE = 10000.0
N_CORES = 8
GROUPS = 4                 # head groups (tensor parallel)
HPC = NH // GROUPS         # heads per core = 4
CPC = HPC * HD             # channels per core = 256
SC = 512                   # seq chunk (matmul free dim)
NSC = S // SC              # 4
NST = S // 128             # 16 s-tiles / k-tiles
KO = HID // 128            # 8 contraction slices for projections
VW = HD + 1                # v row stride per head (64 + ones col)

_cached = None


def _split_waits(nc, mybir, limit=1):
    """This walrus build accepts at most one embedded sync wait per
    instruction; hoist the rest onto NoOps just before it on the same engine."""
    n = 0
    for f in nc.m.functions:
        for b in f.blocks:
            out = []
            changed = False
            for inst in b.instructions:
                si = inst.sync_info
                waits = list(si.on_wait) if (si and si.on_wait) else []
                if len(waits) > limit:
                    keep = waits[-limit:]
                    excess = waits[:-limit]
                    for ci in range(0, len(excess), limit):
                        out.append(mybir.InstNoOp(
                            name=f"{inst.name}-wsplit-{ci}",
                            engine=inst.engine,
                            sync_info=mybir.SyncInfo(
                                on_wait=excess[ci:ci + limit], on_update=[]),
                            bass_nofuse=True,
                        ))
                        n += 1
                    inst.sync_info = mybir.SyncInfo(
                        on_wait=keep,
                        on_update=(list(si.on_update) if si else []))
                    changed = True
                out.append(inst)
            if changed:
                b.instructions = out
    return n


def _build():
    import concourse.bass as bass
    import concourse.mybir as mybir
    import concourse.tile as tile

    f32 = mybir.dt.float32
    bf16 = mybir.dt.bfloat16
    AF = mybir.ActivationFunctionType

    nc = bass.Bass()
    xT = nc.dram_tensor("xT", [HID, S], bf16, kind="ExternalInput")
    wqT = nc.dram_tensor("wqT", [HID, CPC], bf16, kind="ExternalInput")
    wkT = nc.dram_tensor("wkT", [HID, CPC], bf16, kind="ExternalInput")
    wvT = nc.dram_tensor("wvT", [HID, CPC], bf16, kind="ExternalInput")
    woT = nc.dram_tensor("woT", [CPC, HID], bf16, kind="ExternalInput")
    cos2 = nc.dram_tensor("cos2", [128, 2, S], bf16, kind="ExternalInput")
    sin2 = nc.dram_tensor("sin2", [128, 2, S], bf16, kind="ExternalInput")
    out = nc.dram_tensor("out", [S, HID], bf16, kind="ExternalOutput")

    with tile.TileContext(nc) as tc:
        with (
            tc.tile_pool(name="persist", bufs=1) as persist,
            tc.tile_pool(name="pb", bufs=5) as pb,
            tc.tile_pool(name="rope", bufs=3) as rope,
            tc.tile_pool(name="avq", bufs=2) as avq_pool,
            tc.tile_pool(name="ptmp", bufs=2) as ptmp,
            tc.tile_pool(name="pc", bufs=2) as pc,
            tc.tile_pool(name="xw", bufs=1) as xw,
            tc.tile_pool(name="mmp", bufs=2, space="PSUM") as mm_pool,
            tc.tile_pool(name="prj", bufs=1, space="PSUM") as proj_pool,
            tc.tile_pool(name="pop", bufs=2, space="PSUM") as pop_pool,
        ):
            # ---- persistent SBUF ----
            cos_sb = persist.tile([128, 2, S], bf16)
            sin_sb = persist.tile([128, 2, S], bf16)
            wo_sb = persist.tile([128, 2, HID], bf16)
            # [c, s] layout: tile 0 = heads 0,1 (64 rows each); tile 1 = 2,3
            k_cs = [[persist.tile([128, SC], bf16, name=f"kcs{i}_{c}")
                     for c in range(NSC)] for i in range(2)]
            q_cs = [persist.tile([128, S], bf16, name=f"qcs{i}")
                    for i in range(2)]
            v_bf = [persist.tile([128, HPC * VW], bf16, name=f"vbf{t}")
                    for t in range(NST)]
            avt_sb = persist.tile([128, 2, S], bf16)
            bias_sb = persist.tile([128, 1], f32)
            junk_sb = persist.tile([128, 256], bf16)
            ident_sb = persist.tile([128, 128], bf16)
            nc.vector.memset(junk_sb[:], 0.0)
            nc.vector.memset(bias_sb[:], -4.0)
            # identity for tail PE-transposes: 1.0 where col == partition
            nc.vector.memset(ident_sb[:], 1.0)
            nc.gpsimd.affine_select(
                ident_sb[:], ident_sb[:], [[1, 128]],
                mybir.AluOpType.is_equal, 0.0, base=0, channel_multiplier=-1)
            onesv_f = persist.tile([128, HPC], f32)
            nc.vector.memset(onesv_f[:], 1.0)
            for t in range(NST):
                vt_ones = v_bf[t][:].rearrange("p (h w) -> p h w", w=VW)
                nc.vector.tensor_copy(out=vt_ones[:, :, HD], in_=onesv_f[:])

            x_sb = [xw.tile([128, KO, SC], bf16, name=f"x{c}")
                    for c in range(NSC)]
            wk_sb = xw.tile([128, KO, CPC], bf16, name="wk")
            wq_sb = xw.tile([128, KO, CPC], bf16, name="wq")
            wv_sb = xw.tile([128, KO, CPC], bf16, name="wv")

            # ---- PE warm-up: keep the tensor engine continuously busy
            # through the initial DMA wait so the p-state ramp finishes on
            # junk work (cost model: full speed after 3us continuous). ----
            warm_ps = pop_pool.tile([128, SC], f32, tag="bp", name="warm")
            for i in range(37):
                nc.tensor.matmul(
                    warm_ps[:, 0:128],
                    junk_sb[:, 0:128], junk_sb[:, 128:256],
                    start=True, stop=True,
                )

            def dma_w(w_sb, wdram, kos=None):
                # one DMA: the DRAM side is a flat affine pattern
                kos = kos or slice(0, KO)
                nc.sync.dma_start(
                    w_sb[:, kos],
                    wdram[:].rearrange("(o p) c -> p o c", p=128)[:, kos])

            def dma_x(c, kos=None):
                kos = kos or slice(0, KO)
                nc.sync.dma_start(
                    x_sb[c][:, kos],
                    xT[:, c * SC:(c + 1) * SC].rearrange(
                        "(o p) s -> p o s", p=128)[:, kos])

            # DMA priority order: the transfers serialize, so sequence them
            # by first-use time: k0/q0 deps, then x/cos/sin per chunk just
            # ahead of that chunk's projection + RoPE.
            def dma_cs(c):
                sl = slice(c * SC, (c + 1) * SC)
                nc.sync.dma_start(cos_sb[:, :, sl], cos2[:, :, sl])
                nc.sync.dma_start(sin_sb[:, :, sl], sin2[:, :, sl])

            # wk/x0/wq split in ko halves so k0's first matmuls start ~2us
            # earlier (the ko slices are consumed in order); sin0/cos0 land
            # between the wq halves so the k0 RoPE muls are never
            # table-gated.
            dma_w(wk_sb, wkT, slice(0, KO // 2))
            dma_x(0, slice(0, KO // 2))
            dma_w(wk_sb, wkT, slice(KO // 2, KO))
            dma_x(0, slice(KO // 2, KO))
            dma_w(wq_sb, wqT, slice(0, KO // 2))
            nc.sync.dma_start(sin_sb[:, :, 0:SC], sin2[:, :, 0:SC])
            nc.sync.dma_start(cos_sb[:, :, 0:SC], cos2[:, :, 0:SC])
            dma_w(wq_sb, wqT, slice(KO // 2, KO))
            dma_x(1)
            dma_cs(1)
            dma_x(2)
            dma_cs(2)
            dma_x(3)
            dma_w(wv_sb, wvT)
            dma_cs(3)
            for cs in range(2):
                nc.sync.dma_start(wo_sb[:, cs], woT[cs * 128:(cs + 1) * 128, :])

            # ---- projections + RoPE ----
            def qk_thunks(w_sb, dst_cs, c, chunked=False, defer_adds=False,
                          on_mm=False, nmm=4, hh1_dve=False):
                """Projection chunk as a list of thunks: matmul pieces (nmm
                each) + one RoPE piece, so the PE work can interleave
                between score groups without starving ACT. H set (half=1)
                first: the sin-term muls that read it overlap the L half's
                matmuls."""
                st = {}
                order = [(1, ko) for ko in range(KO)] + \
                        [(0, ko) for ko in range(KO)]

                def mm_piece(lo):
                    def f():
                        if "ps" not in st:
                            st["ps"] = (
                                mm_pool.tile([128, 2, SC], f32, tag="mm",
                                             name="qkps")
                                if on_mm else
                                proj_pool.tile([128, 2, SC], f32, tag="proj",
                                               name="qkps"))
                        ps = st["ps"]
                        for half, ko in order[lo:lo + nmm]:
                            nc.tensor.matmul(
                                ps[:, half],
                                w_sb[:, ko, half * 128:(half + 1) * 128],
                                x_sb[c][:, ko, :],
                                start=(ko == 0), stop=(ko == KO - 1),
                            )
                    return f

                def rope_piece():
                    ps = st["ps"]
                    sl = slice(c * SC, (c + 1) * SC)
                    tmc = rope.tile([128, 2, SC], bf16, tag="tmc")
                    tms = rope.tile([128, 2, SC], bf16, tag="tms")
                    nc.vector.tensor_mul(
                        out=tms[:, 0], in0=ps[:, 1], in1=sin_sb[:, 0, sl])
                    nc.vector.tensor_mul(
                        out=tmc[:, 1], in0=ps[:, 1], in1=cos_sb[:, 1, sl])
                    nc.vector.tensor_mul(
                        out=tmc[:, 0], in0=ps[:, 0], in1=cos_sb[:, 0, sl])
                    nc.vector.tensor_mul(
                        out=tms[:, 1], in0=ps[:, 0], in1=sin_sb[:, 1, sl])
                    # add + partition reshuffle in one: out block (t, hh, d)
                    # of the [c, s] layout takes LH partitions 64t+32hh at
                    # pair d. hh=0 rows feed the even-h steps first, so they
                    # go on DVE (fast); hh=1 rows are needed a step later
                    # and go on Pool.
                    for hh in range(2):
                        # defer_adds (k0 only): all adds on Pool so DVE can
                        # start the next chunk's muls immediately; hh1_dve
                        # (k2/k3): hh1 adds on DVE, whose chain finishes
                        # before Pool's -- the hh1 chain bounds step 1
                        eng = nc.vector if (hh == 0 and not defer_adds) \
                            or (hh == 1 and hh1_dve) else nc.gpsimd
                        for t in range(2):
                            sp = 64 * t + 32 * hh
                            for d in range(2):
                                dp = 64 * hh + 32 * d
                                if chunked:
                                    dst = dst_cs[t][c][dp:dp + 32, :]
                                else:
                                    dst = dst_cs[t][dp:dp + 32, sl]
                                eng.tensor_add(
                                    out=dst,
                                    in0=tmc[sp:sp + 32, d, :],
                                    in1=tms[sp:sp + 32, d, :])

                return [mm_piece(lo) for lo in range(0, 2 * KO, nmm)] \
                    + [rope_piece]

            def qk_chunk(w_sb, dst_cs, c, **kw):
                for t in qk_thunks(w_sb, dst_cs, c, nmm=2 * KO, **kw):
                    t()

            def v_tile(st):
                ps = pop_pool.tile([128, CPC], f32, tag="bp", name="pv")
                for ko in range(KO):
                    nc.tensor.matmul(
                        ps[:],
                        x_sb[st // 4][:, ko, (st % 4) * 128:(st % 4 + 1) * 128],
                        wv_sb[:, ko, :],
                        start=(ko == 0), stop=(ko == KO - 1),
                    )
                psv = ps[:].rearrange("p (h e) -> p h e", e=HD)
                vt_v = v_bf[st][:].rearrange("p (h w) -> p h w", w=VW)
                nc.vector.tensor_copy(out=vt_v[:, :, 0:HD], in_=psv[:])

            # ---- attention steps ----
            def score_group(qc, h, g, p_bf):
                cs, pof = h // 2, (h % 2) * HD
                sps = mm_pool.tile([128, 2, SC], f32, tag="mm")
                for kti in range(2):
                    kt = g * 2 + kti
                    nc.tensor.matmul(
                        sps[:, kti],
                        k_cs[cs][kt // 4][pof:pof + HD,
                                          (kt % 4) * 128:(kt % 4 + 1) * 128],
                        q_cs[cs][pof:pof + HD, qc * SC:(qc + 1) * SC],
                        start=True, stop=True,
                    )
                nc.scalar.activation(
                    out=p_bf[:, g * 2:(g + 1) * 2], in_=sps[:],
                    func=AF.Exp, scale=0.125, bias=bias_sb[:],
                )

            def av_finish(qc, h, avb, av_q):
                # normalization stays on DVE: Pool/GPSIMD cannot read PSUM
                hh = h % 2
                avp = avb[:].rearrange("p (a b) -> p a b", b=128)
                rec = ptmp.tile([128, NSC], f32, tag="rec")
                nc.vector.reciprocal(out=rec[:], in_=avp[:, :, HD:HD + 1])
                for qt in range(4):
                    nc.vector.tensor_scalar_mul(
                        out=av_q[:, qt, hh], in0=avp[:, qt, 0:HD],
                        scalar1=rec[:, qt:qt + 1],
                    )

            def avt_transpose(qc, cs, av_q, tail=False):
                # [q, (hh d)] -> [c, q] via the SBUF crossbar (bf16 2-byte).
                # In the tail the ACT queue is idle, so alternate the two
                # HWDGE queues to halve the issue latency.
                for qt in range(4):
                    qo = qc * SC + qt * 128
                    eng = nc.scalar if (tail and qt % 2) else nc.sync
                    eng.dma_start(
                        avt_sb[:, cs, qo:qo + 128],
                        av_q[:, qt].rearrange("p a b -> p (a b)"),
                        transpose=True,
                    )

            def o_st(qc, sti, alt_q=False):
                st = qc * 4 + sti
                o_sb = pc.tile([128, 2, SC], bf16, tag="o_sb")
                for oc in range(2):
                    po = pop_pool.tile([128, SC], f32, tag="bp", name="po")
                    for cs in range(2):
                        nc.tensor.matmul(
                            po[:],
                            avt_sb[:, cs, st * 128:(st + 1) * 128],
                            wo_sb[:, cs, oc * SC:(oc + 1) * SC],
                            start=(cs == 0), stop=(cs == 1),
                        )
                    # Pool/GPSIMD cannot read PSUM; in the tail ACT is idle
                    # and can, so split the copies DVE/ACT there
                    if alt_q and oc == 1:
                        nc.scalar.copy(out=o_sb[:, oc], in_=po[:])
                    else:
                        nc.vector.tensor_copy(out=o_sb[:, oc], in_=po[:])
                # tail stores split per-oc on alternating queues so the
                # last store waits only the last copy; mid-kernel stores
                # stay whole on the software DGE (HWDGE is busy there)
                if alt_q:
                    for oc in range(2):
                        eng = nc.sync if (sti + oc) % 2 else nc.gpsimd
                        eng.dma_start(
                            out[st * 128:(st + 1) * 128,
                                oc * SC:(oc + 1) * SC],
                            o_sb[:, oc])
                else:
                    nc.gpsimd.dma_start(
                        out[st * 128:(st + 1) * 128, :],
                        o_sb[:].rearrange("p a s -> p (a s)"))

            # ---- ramp: k0, q0, k1 so the first score group can run ~15us
            # in while later k chunks interleave between early groups ----
            qk_chunk(wk_sb, k_cs, 0, chunked=True, on_mm=True)
            qk_chunk(wq_sb, q_cs, 0)
            qk_chunk(wk_sb, k_cs, 1, chunked=True, on_mm=True)

            # extras[(step, group)] -> thunks emitted right after that
            # score group's exp. Budgets keep per-step PE work under the
            # ACT step time (~8.3us): scores 3.4 + extras <= ~5us.
            extras = {
                (2, 1): [lambda: v_tile(5)],
                (2, 3): [lambda: v_tile(6)],
                (2, 5): [lambda: v_tile(7)],
                (2, 7): [lambda: v_tile(8), lambda: v_tile(9)],
                (3, 1): [lambda: v_tile(10)],
                (3, 3): [lambda: v_tile(11)],
                (4, 1): [lambda: v_tile(12)],
                (4, 3): [lambda: v_tile(13)],
                (4, 5): [lambda: v_tile(14)],
                (4, 7): [lambda: v_tile(15)],
            }
            # late q chunks interleave as 4-matmul pieces every other
            # group, so ACT never sees a contiguous 3.4us projection block
            for spots, qc_ in ((((3, 0), (3, 2), (3, 4), (3, 6)), 1),
                               (((6, 1), (6, 3), (7, 1), (7, 3)), 2),
                               (((10, 0), (10, 2), (10, 4), (10, 6)), 3)):
                # (placement tuned so no step exceeds the ACT budget)
                th = qk_thunks(wq_sb, q_cs, qc_)
                for pi in range(4):
                    extras.setdefault(spots[pi], []).append(th[pi])
                extras.setdefault(spots[3], []).append(th[4])
            # drains emitted at the end of each step (AV lags 4 steps, then
            # 2 drains/step from step 11 so only step 15's own drain is
            # left for the tail); o-projections split 2 s-tiles at a time
            # to level the per-step PE load.
            # drains spread so every step's PE load stays under the ~8.3us
            # ACT step time, and starting only at step 5 -- their AV pieces
            # read every v tile, so all v_tile emissions (through step 4)
            # must precede them. o-projections run at least one step after
            # the drain that wrote their avt slices.
            drain_at = {5: [0, 1], 6: [2, 3], 7: [4, 5], 8: [6],
                        9: [7, 8], 10: [9], 11: [10], 12: [11], 13: [12],
                        14: [13], 15: [14]}
            o_at = {8: (0, [0, 1, 2, 3]), 11: (1, [0, 1]),
                    12: (1, [2]), 13: (1, [3]), 14: (2, [0, 1]),
                    15: (2, [2, 3])}

            steps = [(qc, h) for qc in range(NSC) for h in range(HPC)]
            hist = {}
            av_q = [None]
            drain_avb = {}

            def drain_thunks(j):
                # one drain = 4 AV q-tile pieces (~0.43us PE each) + a
                # finish (rec+norm+transpose); spread across group slots so
                # ACT never waits behind a contiguous AV block
                def av_piece(qt):
                    def f():
                        (pqc, ph), pp = hist[j]
                        if j not in drain_avb:
                            drain_avb[j] = pop_pool.tile(
                                [128, SC], f32, tag="bp", name="avb")
                        avp = drain_avb[j][:].rearrange(
                            "p (a b) -> p a b", b=128)
                        for kt in range(NST):
                            nc.tensor.matmul(
                                avp[:, qt, 0:VW],
                                pp[:, kt, qt * 128:(qt + 1) * 128],
                                v_bf[kt][:, ph * VW:(ph + 1) * VW],
                                start=(kt == 0), stop=(kt == NST - 1),
                            )
                    return f

                def fin():
                    (pqc, ph), pp = hist.pop(j)
                    avb = drain_avb.pop(j)
                    if ph % 2 == 0:
                        av_q[0] = avq_pool.tile(
                            [128, NSC, 2, HD], bf16, tag="avq", name="av_q")
                    av_finish(pqc, ph, avb, av_q[0])
                    if ph % 2 == 1:
                        avt_transpose(pqc, ph // 2, av_q[0])

                return [av_piece(qt) for qt in range(4)] + [fin]

            # assemble the per-step work queue: group-keyed extras, then
            # drains (their avt feeds this step's o-projections) and o
            # s-tiles distributed over the group slots
            work_at = {}
            for (i, g), ths in extras.items():
                work_at.setdefault(i, {}).setdefault(g, []).extend(ths)
            for i in range(len(steps)):
                # per-step work order: o-projections first (their avt
                # inputs are at least a step old), then older drains, then
                # lag-1 drains (whose AV needs the previous step's LAST
                # exp, landing ~2 exp-slots into this step)
                pending = []
                if i in o_at:
                    oqc, stis = o_at[i]
                    pending.extend(
                        (lambda q=oqc, s=sti: o_st(q, s)) for sti in stis)
                for j in sorted(drain_at.get(i, []), key=lambda j: -(i - j)):
                    pending.extend(drain_thunks(j))
                n = len(pending)
                for k, th in enumerate(pending):
                    g = min(7, 2 + k * 5 // max(n, 1))
                    work_at.setdefault(i, {}).setdefault(g, []).append(th)

            # steps 0 and 1 interleave group-wise: step 1's exps depend only
            # on the Pool-side hh1 RoPE adds, which complete during step
            # 0's DVE rope-chain waits, so they fill step 0's ACT gaps.
            p01 = []
            for i in range(2):
                p_bf = pb.tile([128, NST, SC], bf16, tag="p_bf", name="pbf01")
                hist[i] = (steps[i], p_bf)
                p01.append(p_bf)
            seq01 = [
                ("s", 0, 0), ("k", 2), ("s", 0, 1), ("s", 0, 2), ("s", 0, 3),
                ("k", 3), ("s", 1, 0), ("s", 1, 1), ("v", 0),
                ("s", 0, 4), ("s", 0, 5), ("v", 1), ("s", 1, 2), ("s", 1, 3),
                ("v", 2), ("s", 1, 4), ("s", 1, 5), ("v", 3),
                ("s", 0, 6), ("s", 0, 7), ("v", 4), ("s", 1, 6), ("s", 1, 7),
            ]
            for item in seq01:
                if item[0] == "s":
                    _, si, g = item
                    score_group(0, si, g, p01[si])
                elif item[0] == "k":
                    qk_chunk(wk_sb, k_cs, item[1], chunked=True,
                             hh1_dve=(item[1] == 3))
                else:
                    v_tile(item[1])

            last = len(steps) - 1
            for i, (qc, h) in list(enumerate(steps))[2:]:
                p_bf = pb.tile([128, NST, SC], bf16, tag="p_bf")
                hist[i] = ((qc, h), p_bf)
                for g in range(NST // 2):
                    score_group(qc, h, g, p_bf)
                    for thunk in work_at.get(i, {}).get(g, []):
                        thunk()

            # tail: the last drain runs as a per-q-tile pipeline -- AV,
            # normalize, transpose, o-project, store -- so each q-tile's
            # store starts as soon as its own chain is done
            (_, _), pp = hist.pop(last)
            # two alternating PSUM banks: a start=True matmul re-zeroes its
            # whole 2KB bank region, so staying in one bank would WAR
            # against the previous q-tile's norm reads
            avbs = [pop_pool.tile([128, SC], f32, tag="bp", name=f"avbt{z}")
                    for z in range(2)]
            for qt in range(4):
                avp = avbs[qt % 2][:].rearrange(
                    "p (a b) -> p a b", b=128)[:, qt // 2 * 2]
                for kt in range(NST):
                    nc.tensor.matmul(
                        avp[:, 0:VW],
                        pp[:, kt, qt * 128:(qt + 1) * 128],
                        v_bf[kt][:, (HPC - 1) * VW:HPC * VW],
                        start=(kt == 0), stop=(kt == NST - 1),
                    )
                # norm + transpose overlap the next q-tile's AV matmuls.
                # Transposes run on the idle PE via the identity trick (one
                # per free mm-ring bank -- no zero-region WAR), with the
                # PSUM->SBUF copies split DVE/ACT; this avoids the 4-deep
                # serial HWDGE transpose chain.
                rec = ptmp.tile([128, 1], f32, tag="rec1", name="rec1")
                nc.vector.reciprocal(out=rec[:], in_=avp[:, HD:HD + 1])
                nc.vector.tensor_scalar_mul(
                    out=av_q[0][:, qt, 1], in0=avp[:, 0:HD],
                    scalar1=rec[:],
                )
                trt = mm_pool.tile([128, 128], bf16, tag="mm", name="trt")
                nc.tensor.transpose(
                    trt[:], av_q[0][:, qt].rearrange("p a b -> p (a b)"),
                    ident_sb[:])
                qo = (NSC - 1) * SC + qt * 128
                if qt % 2:
                    nc.scalar.copy(out=avt_sb[:, 1, qo:qo + 128], in_=trt[:])
                else:
                    nc.vector.tensor_copy(
                        out=avt_sb[:, 1, qo:qo + 128], in_=trt[:])
            for qt in range(4):
                o_st(NSC - 1, qt, alt_q=True)

    _split_waits(nc, mybir)
    return nc


def _rope_tables():
    import ml_dtypes
    inv_freq = 1.0 / (BASE ** (np.arange(0, HD, 2, dtype=np.float32) / HD))
    t = np.arange(S, dtype=np.float32)
    freqs = np.einsum("i,j->ij", t, inv_freq)        # [S, 32]
    cos_t = np.cos(freqs).T.astype(np.float32)       # [32, S]
    sin_t = np.sin(freqs).T.astype(np.float32)
    rows = np.arange(128) % 32
    cos2 = np.stack([cos_t[rows], cos_t[rows]], axis=1)      # [128, 2, S]
    sin2 = np.stack([-sin_t[rows], sin_t[rows]], axis=1)
    return (cos2.astype(ml_dtypes.bfloat16), sin2.astype(ml_dtypes.bfloat16))


def _run(inputs, trace=False):
    global _cached
    import ml_dtypes
    from concourse.bass_utils import run_bass_kernel_spmd

    x = np.asarray(inputs["x"], dtype=np.float32)
    wq = np.asarray(inputs["wq"], dtype=np.float32)
    wk = np.asarray(inputs["wk"], dtype=np.float32)
    wv = np.asarray(inputs["wv"], dtype=np.float32)
    wo = np.asarray(inputs["wo"], dtype=np.float32)
    bq = np.asarray(inputs["bq"], dtype=np.float32)
    bk = np.asarray(inputs["bk"], dtype=np.float32)
    bv = np.asarray(inputs["bv"], dtype=np.float32)
    bo = np.asarray(inputs["bo"], dtype=np.float32)
    assert not (bq.any() or bk.any() or bv.any()), \
        "nonzero qkv biases not supported by this kernel build"

    if _cached is None:
        _cached = _build()
    nc = _cached

    cos2, sin2 = _rope_tables()
    # L/H channel order: position p -> head p//32, channel p%32 (+32 for H)
    p = np.arange(128)
    lorder = (p // 32) * HD + (p % 32)
    order = np.concatenate([lorder, lorder + 32])
    bf = ml_dtypes.bfloat16
    in_maps = []
    for core in range(N_CORES):
        b, g = divmod(core, GROUPS)
        cs = slice(g * CPC, (g + 1) * CPC)
        in_maps.append({
            "xT": np.ascontiguousarray(x[b].T).astype(bf),
            "wqT": np.ascontiguousarray(wq[cs][order].T).astype(bf),
            "wkT": np.ascontiguousarray(wk[cs][order].T).astype(bf),
            "wvT": np.ascontiguousarray(wv[cs].T).astype(bf),
            "woT": np.ascontiguousarray(wo[:, cs].T).astype(bf),
            "cos2": cos2,
            "sin2": sin2,
        })

    res = run_bass_kernel_spmd(
        nc, in_maps, core_ids=list(range(N_CORES)), trace=trace)

    outp = np.zeros((B, S, HID), dtype=np.float32)
    for core in range(N_CORES):
        b = core // GROUPS
        outp[b] += res.results[core]["out"].astype(np.float32)
    outp += bo
    return outp, res


def kernel(**inputs):
    outp, _ = _run(inputs, trace=False)
    return outp


# revision 103
# speedup vs baseline: 1.0135x; 1.0135x over previous
"""MultiHeadAttention (B=2, S=2048, HID=1024, NH=16, HD=64, RoPE) on 8 TRN2 cores.

Sharding: 8 cores = 2 batches x 4 head-groups (4 heads per core).
Per core: q/k/v projections for its 4 heads (tensor parallel on H), RoPE,
attention, and a partial o-projection over its 256 channels. Host sums the
4 partial o-projections per batch (the TP unshard) and adds bo.

All compute tensors are bf16 (x, weights, RoPE tables, q/k/v, softmax
weights); matmuls accumulate in f32 PSUM. The partial o-projection is
stored bf16 (the host accumulates the four partials in f32), halving the
output DMA.

RoPE without cross-partition ops: the q/k projection weight columns are
split into an L set (channels 0-31 of each head) and an H set (channels
32-63), so each PSUM partition holds a channel and its rotate-half partner
at the same partition index in two PSUM banks. RoPE is then two full-width
DVE muls (the sin term reads the PSUM pair dim reversed; signs live in the
sin table) plus one Pool add -> bf16 [128, 2, S].

Attention: scores per (head, k-tile) are bf16 matmuls ([64,128] x
[64,512]); exp on ACT with scale=1/8, bias=-4 (the bias cancels in the
row-sum normalization). AV runs transposed -- p [128k,128q] stationary,
v [128k,65] moving (65th col = ones accumulates the row sums) -- packing
the 64-wide head dim into the free axis at full 128-partition occupancy.
Normalization is a per-partition tensor_scalar mul with the reciprocal row
sums; DMA transposes put the normalized attention back in [channel, seq]
for the o-projection.

Schedule (the big lever -- ACT exp is ~133us busy and PE ~140us, so the
span is set by how tightly both pipelines pack):
- PE warm-up: ~30 junk matmuls starting at t~0.3us keep the tensor engine
  continuously busy through the initial input DMAs, so the cost model's
  p-state ramp (0.65/1.2 GHz until 3us of continuous execution) is spent
  on throwaway work and k/q projections run at full 2.4 GHz.
- DMA order follows first-use: wk/x0/wq in ko halves, then per-chunk
  cos/sin and x just ahead of that chunk's projection + RoPE.
- First exp ~19us in: emit k0, q0, k1, then steps 0 and 1 interleaved
  group-wise (step 1's exps depend only on the Pool-side hh1 RoPE adds,
  which complete during step 0's DVE rope-chain waits), with k2/k3 and
  the first v tiles woven between groups.
- All other work is piecewise: late q chunks as 4-matmul pieces, drains
  as per-q-tile AV pieces + a finish, o-projections per s-tile -- all
  distributed over each step's 8 group slots with per-step PE load kept
  just under the ~8.3us ACT step time. Per step: o-projections first
  (their avt inputs are a step old), older drains, then lag-1 drains
  (whose AV needs the previous step's last exp).
- Tail after the last exp: per-q-tile pipeline of AV (two alternating
  PSUM banks -- a start=True matmul re-zeroes its whole 2KB bank, which
  would WAR against the previous q-tile's norm), norm, PE transpose via
  an identity matrix (the DMA-transpose queue serializes at 625ns/issue),
  with PSUM->SBUF copies split DVE/ACT and per-oc stores on alternating
  DMA queues.

PSUM (8 banks): [128,2,512] f32 score-pair ring (tag mm, bufs=2 -> 4
banks, also k0/k1 and the tail transposes), one [128,2,512] projection
slot (tag proj, 2 banks) for all later q/k chunks, and a [128,512] ring
(tag bp, bufs=2 -> 2 banks) shared by warm-up, v-proj, AV accumulators,
and o-proj.

Hardware constraints found the hard way: Pool/GPSIMD cannot touch PSUM
(BIR verifier), only one open PSUM accumulation group per 2KB bank, DMA
transposes only on the SP/ACT hardware DGE queues, and the Tile
scheduler orders a consumer only against producers already emitted (so
emission order is part of correctness, not just performance).
"""

import numpy as np

B, S, HID = 2, 2048, 1024
NH, HD = 16, 64
BASE = 10000.0
N_CORES = 8
GROUPS = 4                 # head groups (tensor parallel)
HPC = NH // GROUPS         # heads per core = 4
CPC = HPC * HD             # channels per core = 256
SC = 512                   # seq chunk (matmul free dim)
NSC = S // SC              # 4
NST = S // 128             # 16 s-tiles / k-tiles
KO = HID // 128            # 8 contraction slices for projections
VW = HD + 1                # v row stride per head (64 + ones col)

_cached = None


def _split_waits(nc, mybir, limit=1):
    """This walrus build accepts at most one embedded sync wait per
    instruction; hoist the rest onto NoOps just before it on the same engine."""
    n = 0
    for f in nc.m.functions:
        for b in f.blocks:
            out = []
            changed = False
            for inst in b.instructions:
                si = inst.sync_info
                waits = list(si.on_wait) if (si and si.on_wait) else []
                if len(waits) > limit:
                    keep = waits[-limit:]
                    excess = waits[:-limit]
                    for ci in range(0, len(excess), limit):
                        out.append(mybir.InstNoOp(
                            name=f"{inst.name}-wsplit-{ci}",
                            engine=inst.engine,
                            sync_info=mybir.SyncInfo(
                                on_wait=excess[ci:ci + limit], on_update=[]),
                            bass_nofuse=True,
                        ))
                        n += 1
                    inst.sync_info = mybir.SyncInfo(
                        on_wait=keep,
                        on_update=(list(si.on_update) if si else []))
                    changed = True
                out.append(inst)
            if changed:
                b.instructions = out
    return n


def _build():
    import concourse.bass as bass
    import concourse.mybir as mybir
    import concourse.tile as tile

    f32 = mybir.dt.float32
    bf16 = mybir.dt.bfloat16
    AF = mybir.ActivationFunctionType

    nc = bass.Bass()
    xT = nc.dram_tensor("xT", [HID, S], bf16, kind="ExternalInput")
    wqT = nc.dram_tensor("wqT", [HID, CPC], bf16, kind="ExternalInput")
    wkT = nc.dram_tensor("wkT", [HID, CPC], bf16, kind="ExternalInput")
    wvT = nc.dram_tensor("wvT", [HID, CPC], bf16, kind="ExternalInput")
    woT = nc.dram_tensor("woT", [CPC, HID], bf16, kind="ExternalInput")
    cos2 = nc.dram_tensor("cos2", [128, 2, S], bf16, kind="ExternalInput")
    sin2 = nc.dram_tensor("sin2", [128, 2, S], bf16, kind="ExternalInput")
    out = nc.dram_tensor("out", [S, HID], bf16, kind="ExternalOutput")

    with tile.TileContext(nc) as tc:
        with (
            tc.tile_pool(name="persist", bufs=1) as persist,
            tc.tile_pool(name="pb", bufs=5) as pb,
            tc.tile_pool(name="rope", bufs=3) as rope,
            tc.tile_pool(name="avq", bufs=2) as avq_pool,
            tc.tile_pool(name="ptmp", bufs=2) as ptmp,
            tc.tile_pool(name="pc", bufs=2) as pc,
            tc.tile_pool(name="xw", bufs=1) as xw,
            tc.tile_pool(name="mmp", bufs=2, space="PSUM") as mm_pool,
            tc.tile_pool(name="prj", bufs=1, space="PSUM") as proj_pool,
            tc.tile_pool(name="pop", bufs=2, space="PSUM") as pop_pool,
        ):
            # ---- persistent SBUF ----
            cos_sb = persist.tile([128, 2, S], bf16)
            sin_sb = persist.tile([128, 2, S], bf16)
            wo_sb = persist.tile([128, 2, HID], bf16)
            # [c, s] layout: tile 0 = heads 0,1 (64 rows each); tile 1 = 2,3
            k_cs = [[persist.tile([128, SC], bf16, name=f"kcs{i}_{c}")
                     for c in range(NSC)] for i in range(2)]
            q_cs = [persist.tile([128, S], bf16, name=f"qcs{i}")
                    for i in range(2)]
            v_bf = [persist.tile([128, HPC * VW], bf16, name=f"vbf{t}")
                    for t in range(NST)]
            avt_sb = persist.tile([128, 2, S], bf16)
            bias_sb = persist.tile([128, 1], f32)
            junk_sb = persist.tile([128, 256], bf16)
            ident_sb = persist.tile([128, 128], bf16)
            nc.vector.memset(junk_sb[:], 0.0)
            nc.vector.memset(bias_sb[:], -4.0)
            # identity for tail PE-transposes: 1.0 where col == partition
            nc.vector.memset(ident_sb[:], 1.0)
            nc.gpsimd.affine_select(
                ident_sb[:], ident_sb[:], [[1, 128]],
                mybir.AluOpType.is_equal, 0.0, base=0, channel_multiplier=-1)
            onesv_f = persist.tile([128, HPC], f32)
            nc.vector.memset(onesv_f[:], 1.0)
            for t in range(NST):
                vt_ones = v_bf[t][:].rearrange("p (h w) -> p h w", w=VW)
                nc.vector.tensor_copy(out=vt_ones[:, :, HD], in_=onesv_f[:])

            x_sb = [xw.tile([128, KO, SC], bf16, name=f"x{c}")
                    for c in range(NSC)]
            wk_sb = xw.tile([128, KO, CPC], bf16, name="wk")
            wq_sb = xw.tile([128, KO, CPC], bf16, name="wq")
            wv_sb = xw.tile([128, KO, CPC], bf16, name="wv")

            # ---- PE warm-up: keep the tensor engine continuously busy
            # through the initial DMA wait so the p-state ramp finishes on
            # junk work (cost model: full speed after 3us continuous). ----
            warm_ps = pop_pool.tile([128, SC], f32, tag="bp", name="warm")
            for i in range(37):
                nc.tensor.matmul(
                    warm_ps[:, 0:128],
                    junk_sb[:, 0:128], junk_sb[:, 128:256],
                    start=True, stop=True,
                )

            def dma_w(w_sb, wdram, kos=None):
                # one DMA: the DRAM side is a flat affine pattern
                kos = kos or slice(0, KO)
                nc.sync.dma_start(
                    w_sb[:, kos],
                    wdram[:].rearrange("(o p) c -> p o c", p=128)[:, kos])

            def dma_x(c, kos=None):
                kos = kos or slice(0, KO)
                nc.sync.dma_start(
                    x_sb[c][:, kos],
                    xT[:, c * SC:(c + 1) * SC].rearrange(
                        "(o p) s -> p o s", p=128)[:, kos])

            # DMA priority order: the transfers serialize, so sequence them
            # by first-use time: k0/q0 deps, then x/cos/sin per chunk just
            # ahead of that chunk's projection + RoPE.
            def dma_cs(c):
                sl = slice(c * SC, (c + 1) * SC)
                nc.sync.dma_start(cos_sb[:, :, sl], cos2[:, :, sl])
                nc.sync.dma_start(sin_sb[:, :, sl], sin2[:, :, sl])

            # wk/x0/wq split in ko halves so k0's first matmuls start ~2us
            # earlier (the ko slices are consumed in order); sin0/cos0 land
            # between the wq halves so the k0 RoPE muls are never
            # table-gated.
            dma_w(wk_sb, wkT, slice(0, KO // 2))
            dma_x(0, slice(0, KO // 2))
            dma_w(wk_sb, wkT, slice(KO // 2, KO))
            dma_x(0, slice(KO // 2, KO))
            nc.sync.dma_start(sin_sb[:, :, 0:SC], sin2[:, :, 0:SC])
            nc.sync.dma_start(cos_sb[:, :, 0:SC], cos2[:, :, 0:SC])
            dma_w(wq_sb, wqT, slice(0, KO // 2))
            dma_w(wq_sb, wqT, slice(KO // 2, KO))
            dma_x(1)
            dma_cs(1)
            dma_x(2)
            dma_cs(2)
            dma_x(3)
            dma_w(wv_sb, wvT)
            dma_cs(3)
            for cs in range(2):
                nc.sync.dma_start(wo_sb[:, cs], woT[cs * 128:(cs + 1) * 128, :])

            # ---- projections + RoPE ----
            def qk_thunks(w_sb, dst_cs, c, chunked=False, defer_adds=False,
                          on_mm=False, nmm=4, hh1_dve=False):
                """Projection chunk as a list of thunks: matmul pieces (nmm
                each) + one RoPE piece, so the PE work can interleave
                between score groups without starving ACT. H set (half=1)
                first: the sin-term muls that read it overlap the L half's
                matmuls."""
                st = {}
                order = [(1, ko) for ko in range(KO)] + \
                        [(0, ko) for ko in range(KO)]

                def mm_piece(lo):
                    def f():
                        if "ps1" not in st:
                            # one PSUM tile PER HALF: tile-granular
                            # dependency tracking means the H-half RoPE
                            # muls would otherwise wait for the L half too
                            st["ps1"] = proj_pool.tile(
                                [128, SC], f32, tag="projH", name="qkpsh")
                            st["ps0"] = proj_pool.tile(
                                [128, SC], f32, tag="projL", name="qkpsl")
                        for half, ko in order[lo:lo + nmm]:
                            nc.tensor.matmul(
                                st[f"ps{half}"][:],
                                w_sb[:, ko, half * 128:(half + 1) * 128],
                                x_sb[c][:, ko, :],
                                start=(ko == 0), stop=(ko == KO - 1),
                            )
                    return f

                def rope_piece():
                    ps1, ps0 = st["ps1"], st["ps0"]
                    sl = slice(c * SC, (c + 1) * SC)
                    tmc = rope.tile([128, 2, SC], bf16, tag="tmc")
                    tms = rope.tile([128, 2, SC], bf16, tag="tms")
                    nc.vector.tensor_mul(
                        out=tms[:, 0], in0=ps1[:], in1=sin_sb[:, 0, sl])
                    nc.vector.tensor_mul(
                        out=tmc[:, 1], in0=ps1[:], in1=cos_sb[:, 1, sl])
                    nc.vector.tensor_mul(
                        out=tmc[:, 0], in0=ps0[:], in1=cos_sb[:, 0, sl])
                    nc.vector.tensor_mul(
                        out=tms[:, 1], in0=ps0[:], in1=sin_sb[:, 1, sl])
                    # add + partition reshuffle in one: out block (t, hh, d)
                    # of the [c, s] layout takes LH partitions 64t+32hh at
                    # pair d. hh=0 rows feed the even-h steps first, so they
                    # go on DVE (fast); hh=1 rows are needed a step later
                    # and go on Pool.
                    for hh in range(2):
                        # defer_adds (k0 only): all adds on Pool so DVE can
                        # start the next chunk's muls immediately; hh1_dve
                        # (k2/k3): hh1 adds on DVE, whose chain finishes
                        # before Pool's -- the hh1 chain bounds step 1
                        eng = nc.vector if (hh == 0 and not defer_adds) \
                            or (hh == 1 and hh1_dve) else nc.gpsimd
                        for t in range(2):
                            sp = 64 * t + 32 * hh
                            for d in range(2):
                                dp = 64 * hh + 32 * d
                                if chunked:
                                    dst = dst_cs[t][c][dp:dp + 32, :]
                                else:
                                    dst = dst_cs[t][dp:dp + 32, sl]
                                eng.tensor_add(
                                    out=dst,
                                    in0=tmc[sp:sp + 32, d, :],
                                    in1=tms[sp:sp + 32, d, :])

                return [mm_piece(lo) for lo in range(0, 2 * KO, nmm)] \
                    + [rope_piece]

            def qk_chunk(w_sb, dst_cs, c, **kw):
                for t in qk_thunks(w_sb, dst_cs, c, nmm=2 * KO, **kw):
                    t()

            def v_tile(st):
                ps = pop_pool.tile([128, CPC], f32, tag="bp", name="pv")
                for ko in range(KO):
                    nc.tensor.matmul(
                        ps[:],
                        x_sb[st // 4][:, ko, (st % 4) * 128:(st % 4 + 1) * 128],
                        wv_sb[:, ko, :],
                        start=(ko == 0), stop=(ko == KO - 1),
                    )
                psv = ps[:].rearrange("p (h e) -> p h e", e=HD)
                vt_v = v_bf[st][:].rearrange("p (h w) -> p h w", w=VW)
                nc.vector.tensor_copy(out=vt_v[:, :, 0:HD], in_=psv[:])

            # ---- attention steps ----
            def score_group(qc, h, g, p_bf):
                cs, pof = h // 2, (h % 2) * HD
                sps = mm_pool.tile([128, 2, SC], f32, tag="mm")
                for kti in range(2):
                    kt = g * 2 + kti
                    nc.tensor.matmul(
                        sps[:, kti],
                        k_cs[cs][kt // 4][pof:pof + HD,
                                          (kt % 4) * 128:(kt % 4 + 1) * 128],
                        q_cs[cs][pof:pof + HD, qc * SC:(qc + 1) * SC],
                        start=True, stop=True,
                    )
                nc.scalar.activation(
                    out=p_bf[:, g * 2:(g + 1) * 2], in_=sps[:],
                    func=AF.Exp, scale=0.125, bias=bias_sb[:],
                )

            def av_finish(qc, h, avb, av_q):
                # normalization stays on DVE: Pool/GPSIMD cannot read PSUM
                hh = h % 2
                avp = avb[:].rearrange("p (a b) -> p a b", b=128)
                rec = ptmp.tile([128, NSC], f32, tag="rec")
                nc.vector.reciprocal(out=rec[:], in_=avp[:, :, HD:HD + 1])
                for qt in range(4):
                    nc.vector.tensor_scalar_mul(
                        out=av_q[:, qt, hh], in0=avp[:, qt, 0:HD],
                        scalar1=rec[:, qt:qt + 1],
                    )

            def avt_transpose(qc, cs, av_q, tail=False):
                # [q, (hh d)] -> [c, q] via the SBUF crossbar (bf16 2-byte).
                # In the tail the ACT queue is idle, so alternate the two
                # HWDGE queues to halve the issue latency.
                for qt in range(4):
                    qo = qc * SC + qt * 128
                    eng = nc.scalar if (tail and qt % 2) else nc.sync
                    eng.dma_start(
                        avt_sb[:, cs, qo:qo + 128],
                        av_q[:, qt].rearrange("p a b -> p (a b)"),
                        transpose=True,
                    )

            def o_st(qc, sti, alt_q=False):
                st = qc * 4 + sti
                o_sb = pc.tile([128, 2, SC], bf16, tag="o_sb")
                for oc in range(2):
                    po = pop_pool.tile([128, SC], f32, tag="bp", name="po")
                    for cs in range(2):
                        nc.tensor.matmul(
                            po[:],
                            avt_sb[:, cs, st * 128:(st + 1) * 128],
                            wo_sb[:, cs, oc * SC:(oc + 1) * SC],
                            start=(cs == 0), stop=(cs == 1),
                        )
                    # Pool/GPSIMD cannot read PSUM; in the tail ACT is idle
                    # and can, so split the copies DVE/ACT there
                    if alt_q and oc == 1:
                        nc.scalar.copy(out=o_sb[:, oc], in_=po[:])
                    else:
                        nc.vector.tensor_copy(out=o_sb[:, oc], in_=po[:])
                # tail stores split per-oc on alternating queues so the
                # last store waits only the last copy; mid-kernel stores
                # stay whole on the software DGE (HWDGE is busy there)
                if alt_q:
                    for oc in range(2):
                        eng = nc.sync if (sti + oc) % 2 else nc.gpsimd
                        eng.dma_start(
                            out[st * 128:(st + 1) * 128,
                                oc * SC:(oc + 1) * SC],
                            o_sb[:, oc])
                else:
                    nc.gpsimd.dma_start(
                        out[st * 128:(st + 1) * 128, :],
                        o_sb[:].rearrange("p a s -> p (a s)"))

            # ---- ramp: k0, q0, k1 so the first score group can run ~15us
            # in while later k chunks interleave between early groups ----
            qk_chunk(wk_sb, k_cs, 0, chunked=True, on_mm=True)
            qk_chunk(wq_sb, q_cs, 0)
            qk_chunk(wk_sb, k_cs, 1, chunked=True, on_mm=True)

            # extras[(step, group)] -> thunks emitted right after that
            # score group's exp. Budgets keep per-step PE work under the
            # ACT step time (~8.3us): scores 3.4 + extras <= ~5us.
            extras = {
                (2, 1): [lambda: v_tile(5)],
                (2, 3): [lambda: v_tile(6)],
                (2, 5): [lambda: v_tile(7)],
                (2, 7): [lambda: v_tile(8), lambda: v_tile(9)],
                (3, 1): [lambda: v_tile(10)],
                (3, 3): [lambda: v_tile(11)],
                (4, 1): [lambda: v_tile(12)],
                (4, 3): [lambda: v_tile(13)],
                (4, 5): [lambda: v_tile(14)],
                (4, 7): [lambda: v_tile(15)],
            }
            # late q chunks interleave as 4-matmul pieces every other
            # group, so ACT never sees a contiguous 3.4us projection block
            for spots, qc_ in ((((3, 0), (3, 2), (3, 4), (3, 6)), 1),
                               (((6, 1), (6, 3), (7, 1), (7, 3)), 2),
                               (((10, 0), (10, 2), (10, 4), (10, 6)), 3)):
                # (placement tuned so no step exceeds the ACT budget)
                th = qk_thunks(wq_sb, q_cs, qc_)
                for pi in range(4):
                    extras.setdefault(spots[pi], []).append(th[pi])
                extras.setdefault(spots[3], []).append(th[4])
            # drains emitted at the end of each step (AV lags 4 steps, then
            # 2 drains/step from step 11 so only step 15's own drain is
            # left for the tail); o-projections split 2 s-tiles at a time
            # to level the per-step PE load.
            # drains spread so every step's PE load stays under the ~8.3us
            # ACT step time, and starting only at step 5 -- their AV pieces
            # read every v tile, so all v_tile emissions (through step 4)
            # must precede them. o-projections run at least one step after
            # the drain that wrote their avt slices.
            drain_at = {5: [0, 1], 6: [2, 3], 7: [4, 5], 8: [6],
                        9: [7, 8], 10: [9], 11: [10], 12: [11], 13: [12],
                        14: [13], 15: [14]}
            o_at = {8: (0, [0, 1, 2, 3]), 11: (1, [0, 1]),
                    12: (1, [2]), 13: (1, [3]), 14: (2, [0, 1]),
                    15: (2, [2, 3])}

            steps = [(qc, h) for qc in range(NSC) for h in range(HPC)]
            hist = {}
            av_q = [None]
            drain_avb = {}

            def drain_thunks(j):
                # one drain = 4 AV q-tile pieces (~0.43us PE each) + a
                # finish (rec+norm+transpose); spread across group slots so
                # ACT never waits behind a contiguous AV block
                def av_piece(qt):
                    def f():
                        (pqc, ph), pp = hist[j]
                        if j not in drain_avb:
                            drain_avb[j] = pop_pool.tile(
                                [128, SC], f32, tag="bp", name="avb")
                        avp = drain_avb[j][:].rearrange(
                            "p (a b) -> p a b", b=128)
                        for kt in range(NST):
                            nc.tensor.matmul(
                                avp[:, qt, 0:VW],
                                pp[:, kt, qt * 128:(qt + 1) * 128],
                                v_bf[kt][:, ph * VW:(ph + 1) * VW],
                                start=(kt == 0), stop=(kt == NST - 1),
                            )
                    return f

                def fin():
                    (pqc, ph), pp = hist.pop(j)
                    avb = drain_avb.pop(j)
                    if ph % 2 == 0:
                        av_q[0] = avq_pool.tile(
                            [128, NSC, 2, HD], bf16, tag="avq", name="av_q")
                    av_finish(pqc, ph, avb, av_q[0])
                    if ph % 2 == 1:
                        avt_transpose(pqc, ph // 2, av_q[0])

                return [av_piece(qt) for qt in range(4)] + [fin]

            # assemble the per-step work queue: group-keyed extras, then
            # drains (their avt feeds this step's o-projections) and o
            # s-tiles distributed over the group slots
            work_at = {}
            for (i, g), ths in extras.items():
                work_at.setdefault(i, {}).setdefault(g, []).extend(ths)
            for i in range(len(steps)):
                # per-step work order: o-projections first (their avt
                # inputs are at least a step old), then older drains, then
                # lag-1 drains (whose AV needs the previous step's LAST
                # exp, landing ~2 exp-slots into this step)
                pending = []
                if i in o_at:
                    oqc, stis = o_at[i]
                    pending.extend(
                        (lambda q=oqc, s=sti: o_st(q, s)) for sti in stis)
                for j in sorted(drain_at.get(i, []), key=lambda j: -(i - j)):
                    pending.extend(drain_thunks(j))
                n = len(pending)
                for k, th in enumerate(pending):
                    g = min(7, 2 + k * 5 // max(n, 1))
                    work_at.setdefault(i, {}).setdefault(g, []).append(th)

            # steps 0 and 1 interleave group-wise: step 1's exps depend only
            # on the Pool-side hh1 RoPE adds, which complete during step
            # 0's DVE rope-chain waits, so they fill step 0's ACT gaps.
            p01 = []
            for i in range(2):
                p_bf = pb.tile([128, NST, SC], bf16, tag="p_bf", name="pbf01")
                hist[i] = (steps[i], p_bf)
                p01.append(p_bf)
            seq01 = [
                ("s", 0, 0), ("k", 2), ("s", 0, 1), ("s", 1, 0), ("s", 1, 1),
                ("s", 0, 2), ("s", 0, 3),
                ("k", 3), ("v", 0),
                ("s", 1, 2), ("s", 1, 3), ("v", 1), ("s", 0, 4), ("s", 0, 5),
                ("v", 2), ("s", 1, 4), ("s", 1, 5), ("v", 3),
                ("s", 0, 6), ("s", 0, 7), ("v", 4), ("s", 1, 6), ("s", 1, 7),
            ]
            for item in seq01:
                if item[0] == "s":
                    _, si, g = item
                    score_group(0, si, g, p01[si])
                elif item[0] == "k":
                    qk_chunk(wk_sb, k_cs, item[1], chunked=True,
                             hh1_dve=(item[1] == 3))
                else:
                    v_tile(item[1])

            last = len(steps) - 1
            for i, (qc, h) in list(enumerate(steps))[2:]:
                p_bf = pb.tile([128, NST, SC], bf16, tag="p_bf")
                hist[i] = ((qc, h), p_bf)
                for g in range(NST // 2):
                    score_group(qc, h, g, p_bf)
                    for thunk in work_at.get(i, {}).get(g, []):
                        thunk()

            # tail: the last drain runs as a per-q-tile pipeline -- AV,
            # normalize, transpose, o-project, store -- so each q-tile's
            # store starts as soon as its own chain is done
            (_, _), pp = hist.pop(last)
            # two alternating PSUM banks: a start=True matmul re-zeroes its
            # whole 2KB bank region, so staying in one bank would WAR
            # against the previous q-tile's norm reads
            avbs = [pop_pool.tile([128, SC], f32, tag="bp", name=f"avbt{z}")
                    for z in range(2)]
            for qt in range(4):
                avp = avbs[qt % 2][:].rearrange(
                    "p (a b) -> p a b", b=128)[:, qt // 2 * 2]
                for kt in range(NST):
                    nc.tensor.matmul(
                        avp[:, 0:VW],
                        pp[:, kt, qt * 128:(qt + 1) * 128],
                        v_bf[kt][:, (HPC - 1) * VW:HPC * VW],
                        start=(kt == 0), stop=(kt == NST - 1),
                    )
                # norm + transpose overlap the next q-tile's AV matmuls.
                # Transposes run on the idle PE via the identity trick (one
                # per free mm-ring bank -- no zero-region WAR), with the
                # PSUM->SBUF copies split DVE/ACT; this avoids the 4-deep
                # serial HWDGE transpose chain.
                rec = ptmp.tile([128, 1], f32, tag="rec1", name="rec1")
                nc.vector.reciprocal(out=rec[:], in_=avp[:, HD:HD + 1])
                nc.vector.tensor_scalar_mul(
                    out=av_q[0][:, qt, 1], in0=avp[:, 0:HD],
                    scalar1=rec[:],
                )
                trt = mm_pool.tile([128, 128], bf16, tag="mm", name="trt")
                nc.tensor.transpose(
                    trt[:], av_q[0][:, qt].rearrange("p a b -> p (a b)"),
                    ident_sb[:])
                qo = (NSC - 1) * SC + qt * 128
                if qt % 2:
                    nc.scalar.copy(out=avt_sb[:, 1, qo:qo + 128], in_=trt[:])
                else:
                    nc.vector.tensor_copy(
                        out=avt_sb[:, 1, qo:qo + 128], in_=trt[:])
            for qt in range(4):
                o_st(NSC - 1, qt, alt_q=True)

    _split_waits(nc, mybir)
    return nc


def _rope_tables():
    import ml_dtypes
    inv_freq = 1.0 / (BASE ** (np.arange(0, HD, 2, dtype=np.float32) / HD))
    t = np.arange(S, dtype=np.float32)
    freqs = np.einsum("i,j->ij", t, inv_freq)        # [S, 32]
    cos_t = np.cos(freqs).T.astype(np.float32)       # [32, S]
    sin_t = np.sin(freqs).T.astype(np.float32)
    rows = np.arange(128) % 32
    cos2 = np.stack([cos_t[rows], cos_t[rows]], axis=1)      # [128, 2, S]
    sin2 = np.stack([-sin_t[rows], sin_t[rows]], axis=1)
    return (cos2.astype(ml_dtypes.bfloat16), sin2.astype(ml_dtypes.bfloat16))


def _run(inputs, trace=False):
    global _cached
    import ml_dtypes
    from concourse.bass_utils import run_bass_kernel_spmd

    x = np.asarray(inputs["x"], dtype=np.float32)
    wq = np.asarray(inputs["wq"], dtype=np.float32)
    wk = np.asarray(inputs["wk"], dtype=np.float32)
    wv = np.asarray(inputs["wv"], dtype=np.float32)
    wo = np.asarray(inputs["wo"], dtype=np.float32)
    bq = np.asarray(inputs["bq"], dtype=np.float32)
    bk = np.asarray(inputs["bk"], dtype=np.float32)
    bv = np.asarray(inputs["bv"], dtype=np.float32)
    bo = np.asarray(inputs["bo"], dtype=np.float32)
    assert not (bq.any() or bk.any() or bv.any()), \
        "nonzero qkv biases not supported by this kernel build"

    if _cached is None:
        _cached = _build()
    nc = _cached

    cos2, sin2 = _rope_tables()
    # L/H channel order: position p -> head p//32, channel p%32 (+32 for H)
    p = np.arange(128)
    lorder = (p // 32) * HD + (p % 32)
    order = np.concatenate([lorder, lorder + 32])
    bf = ml_dtypes.bfloat16
    in_maps = []
    for core in range(N_CORES):
        b, g = divmod(core, GROUPS)
        cs = slice(g * CPC, (g + 1) * CPC)
        in_maps.append({
            "xT": np.ascontiguousarray(x[b].T).astype(bf),
            "wqT": np.ascontiguousarray(wq[cs][order].T).astype(bf),
            "wkT": np.ascontiguousarray(wk[cs][order].T).astype(bf),
            "wvT": np.ascontiguousarray(wv[cs].T).astype(bf),
            "woT": np.ascontiguousarray(wo[:, cs].T).astype(bf),
            "cos2": cos2,
            "sin2": sin2,
        })

    res = run_bass_kernel_spmd(
        nc, in_maps, core_ids=list(range(N_CORES)), trace=trace)

    outp = np.zeros((B, S, HID), dtype=np.float32)
    for core in range(N_CORES):
        b = core // GROUPS
        outp[b] += res.results[core]["out"].astype(np.float32)
    outp += bo
    return outp, res


def kernel(**inputs):
    outp, _ = _run(inputs, trace=False)
    return outp


# revision 108
# speedup vs baseline: 1.0146x; 1.0011x over previous
"""MultiHeadAttention (B=2, S=2048, HID=1024, NH=16, HD=64, RoPE) on 8 TRN2 cores.

Sharding: 8 cores = 2 batches x 4 head-groups (4 heads per core).
Per core: q/k/v projections for its 4 heads (tensor parallel on H), RoPE,
attention, and a partial o-projection over its 256 channels. Host sums the
4 partial o-projections per batch (the TP unshard) and adds bo.

All compute tensors are bf16 (x, weights, RoPE tables, q/k/v, softmax
weights); matmuls accumulate in f32 PSUM. The partial o-projection is
stored bf16 (the host accumulates the four partials in f32), halving the
output DMA.

RoPE without cross-partition ops: the q/k projection weight columns are
split into an L set (channels 0-31 of each head) and an H set (channels
32-63), so each PSUM partition holds a channel and its rotate-half partner
at the same partition index in two PSUM banks. RoPE is then two full-width
DVE muls (the sin term reads the PSUM pair dim reversed; signs live in the
sin table) plus one Pool add -> bf16 [128, 2, S].

Attention: scores per (head, k-tile) are bf16 matmuls ([64,128] x
[64,512]); exp on ACT with scale=1/8, bias=-4 (the bias cancels in the
row-sum normalization). AV runs transposed -- p [128k,128q] stationary,
v [128k,65] moving (65th col = ones accumulates the row sums) -- packing
the 64-wide head dim into the free axis at full 128-partition occupancy.
Normalization is a per-partition tensor_scalar mul with the reciprocal row
sums; DMA transposes put the normalized attention back in [channel, seq]
for the o-projection.

Schedule (the big lever -- ACT exp is ~133us busy and PE ~140us, so the
span is set by how tightly both pipelines pack):
- PE warm-up: ~30 junk matmuls starting at t~0.3us keep the tensor engine
  continuously busy through the initial input DMAs, so the cost model's
  p-state ramp (0.65/1.2 GHz until 3us of continuous execution) is spent
  on throwaway work and k/q projections run at full 2.4 GHz.
- DMA order follows first-use: wk/x0/wq in ko halves, then per-chunk
  cos/sin and x just ahead of that chunk's projection + RoPE.
- First exp ~19us in: emit k0, q0, k1, then steps 0 and 1 interleaved
  group-wise (step 1's exps depend only on the Pool-side hh1 RoPE adds,
  which complete during step 0's DVE rope-chain waits), with k2/k3 and
  the first v tiles woven between groups.
- All other work is piecewise: late q chunks as 4-matmul pieces, drains
  as per-q-tile AV pieces + a finish, o-projections per s-tile -- all
  distributed over each step's 8 group slots with per-step PE load kept
  just under the ~8.3us ACT step time. Per step: o-projections first
  (their avt inputs are a step old), older drains, then lag-1 drains
  (whose AV needs the previous step's last exp).
- Tail after the last exp: per-q-tile pipeline of AV (two alternating
  PSUM banks -- a start=True matmul re-zeroes its whole 2KB bank, which
  would WAR against the previous q-tile's norm), norm, PE transpose via
  an identity matrix (the DMA-transpose queue serializes at 625ns/issue),
  with PSUM->SBUF copies split DVE/ACT and per-oc stores on alternating
  DMA queues.

PSUM (8 banks): [128,2,512] f32 score-pair ring (tag mm, bufs=2 -> 4
banks, also the tail transposes and AV accumulators' neighbors), two
single-bank projection slots (tags projH/projL) -- one PSUM tile PER
HALF of each q/k chunk, because dependency tracking is tile-granular
and a shared tile would make the H-half RoPE muls wait for the L half
too -- and a [128,512] ring (tag bp, bufs=2 -> 2 banks) shared by
warm-up, v-proj, AV accumulators, and o-proj.

Hardware constraints found the hard way: Pool/GPSIMD cannot touch PSUM
(BIR verifier), only one open PSUM accumulation group per 2KB bank,
PSUM-tile dependencies are whole-tile (split tiles to overlap), DMA
transposes only on the SP/ACT hardware DGE queues, and the Tile
scheduler orders a consumer only against producers already emitted (so
emission order is part of correctness, not just performance).
"""

import numpy as np

B, S, HID = 2, 2048, 1024
NH, HD = 16, 64
BASE = 10000.0
N_CORES = 8
GROUPS = 4                 # head groups (tensor parallel)
HPC = NH // GROUPS         # heads per core = 4
CPC = HPC * HD             # channels per core = 256
SC = 512                   # seq chunk (matmul free dim)
NSC = S // SC              # 4
NST = S // 128             # 16 s-tiles / k-tiles
KO = HID // 128            # 8 contraction slices for projections
VW = HD + 1                # v row stride per head (64 + ones col)

_cached = None


def _split_waits(nc, mybir, limit=1):
    """This walrus build accepts at most one embedded sync wait per
    instruction; hoist the rest onto NoOps just before it on the same engine."""
    n = 0
    for f in nc.m.functions:
        for b in f.blocks:
            out = []
            changed = False
            for inst in b.instructions:
                si = inst.sync_info
                waits = list(si.on_wait) if (si and si.on_wait) else []
                if len(waits) > limit:
                    keep = waits[-limit:]
                    excess = waits[:-limit]
                    for ci in range(0, len(excess), limit):
                        out.append(mybir.InstNoOp(
                            name=f"{inst.name}-wsplit-{ci}",
                            engine=inst.engine,
                            sync_info=mybir.SyncInfo(
                                on_wait=excess[ci:ci + limit], on_update=[]),
                            bass_nofuse=True,
                        ))
                        n += 1
                    inst.sync_info = mybir.SyncInfo(
                        on_wait=keep,
                        on_update=(list(si.on_update) if si else []))
                    changed = True
                out.append(inst)
            if changed:
                b.instructions = out
    return n


def _build():
    import concourse.bass as bass
    import concourse.mybir as mybir
    import concourse.tile as tile

    f32 = mybir.dt.float32
    bf16 = mybir.dt.bfloat16
    AF = mybir.ActivationFunctionType

    nc = bass.Bass()
    xT = nc.dram_tensor("xT", [HID, S], bf16, kind="ExternalInput")
    wqT = nc.dram_tensor("wqT", [HID, CPC], bf16, kind="ExternalInput")
    wkT = nc.dram_tensor("wkT", [HID, CPC], bf16, kind="ExternalInput")
    wvT = nc.dram_tensor("wvT", [HID, CPC], bf16, kind="ExternalInput")
    woT = nc.dram_tensor("woT", [CPC, HID], bf16, kind="ExternalInput")
    cos2 = nc.dram_tensor("cos2", [128, 2, S], bf16, kind="ExternalInput")
    sin2 = nc.dram_tensor("sin2", [128, 2, S], bf16, kind="ExternalInput")
    out = nc.dram_tensor("out", [S, HID], bf16, kind="ExternalOutput")

    with tile.TileContext(nc) as tc:
        with (
            tc.tile_pool(name="persist", bufs=1) as persist,
            tc.tile_pool(name="pb", bufs=5) as pb,
            tc.tile_pool(name="rope", bufs=3) as rope,
            tc.tile_pool(name="avq", bufs=2) as avq_pool,
            tc.tile_pool(name="ptmp", bufs=2) as ptmp,
            tc.tile_pool(name="pc", bufs=2) as pc,
            tc.tile_pool(name="xw", bufs=1) as xw,
            tc.tile_pool(name="mmp", bufs=2, space="PSUM") as mm_pool,
            tc.tile_pool(name="prj", bufs=1, space="PSUM") as proj_pool,
            tc.tile_pool(name="pop", bufs=2, space="PSUM") as pop_pool,
        ):
            # ---- persistent SBUF ----
            cos_sb = persist.tile([128, 2, S], bf16)
            sin_sb = persist.tile([128, 2, S], bf16)
            wo_sb = persist.tile([128, 2, HID], bf16)
            # [c, s] layout: tile 0 = heads 0,1 (64 rows each); tile 1 = 2,3
            k_cs = [[persist.tile([128, SC], bf16, name=f"kcs{i}_{c}")
                     for c in range(NSC)] for i in range(2)]
            q_cs = [persist.tile([128, S], bf16, name=f"qcs{i}")
                    for i in range(2)]
            v_bf = [persist.tile([128, HPC * VW], bf16, name=f"vbf{t}")
                    for t in range(NST)]
            avt_sb = persist.tile([128, 2, S], bf16)
            bias_sb = persist.tile([128, 1], f32)
            junk_sb = persist.tile([128, 256], bf16)
            ident_sb = persist.tile([128, 128], bf16)
            nc.vector.memset(junk_sb[:], 0.0)
            nc.vector.memset(bias_sb[:], -4.0)
            # identity for tail PE-transposes: 1.0 where col == partition
            nc.vector.memset(ident_sb[:], 1.0)
            nc.gpsimd.affine_select(
                ident_sb[:], ident_sb[:], [[1, 128]],
                mybir.AluOpType.is_equal, 0.0, base=0, channel_multiplier=-1)
            onesv_f = persist.tile([128, HPC], f32)
            nc.vector.memset(onesv_f[:], 1.0)
            for t in range(NST):
                vt_ones = v_bf[t][:].rearrange("p (h w) -> p h w", w=VW)
                nc.vector.tensor_copy(out=vt_ones[:, :, HD], in_=onesv_f[:])

            x_sb = [xw.tile([128, KO, SC], bf16, name=f"x{c}")
                    for c in range(NSC)]
            wk_sb = xw.tile([128, KO, CPC], bf16, name="wk")
            wq_sb = xw.tile([128, KO, CPC], bf16, name="wq")
            wv_sb = xw.tile([128, KO, CPC], bf16, name="wv")

            # ---- PE warm-up: keep the tensor engine continuously busy
            # through the initial DMA wait so the p-state ramp finishes on
            # junk work (cost model: full speed after 3us continuous). ----
            warm_ps = pop_pool.tile([128, SC], f32, tag="bp", name="warm")
            for i in range(37):
                nc.tensor.matmul(
                    warm_ps[:, 0:128],
                    junk_sb[:, 0:128], junk_sb[:, 128:256],
                    start=True, stop=True,
                )

            def dma_w(w_sb, wdram, kos=None):
                # one DMA: the DRAM side is a flat affine pattern
                kos = kos or slice(0, KO)
                nc.sync.dma_start(
                    w_sb[:, kos],
                    wdram[:].rearrange("(o p) c -> p o c", p=128)[:, kos])

            def dma_x(c, kos=None):
                kos = kos or slice(0, KO)
                nc.sync.dma_start(
                    x_sb[c][:, kos],
                    xT[:, c * SC:(c + 1) * SC].rearrange(
                        "(o p) s -> p o s", p=128)[:, kos])

            # DMA priority order: the transfers serialize, so sequence them
            # by first-use time: k0/q0 deps, then x/cos/sin per chunk just
            # ahead of that chunk's projection + RoPE.
            def dma_cs(c):
                sl = slice(c * SC, (c + 1) * SC)
                nc.sync.dma_start(cos_sb[:, :, sl], cos2[:, :, sl])
                nc.sync.dma_start(sin_sb[:, :, sl], sin2[:, :, sl])

            # wk/x0/wq split in ko halves so k0's first matmuls start ~2us
            # earlier (the ko slices are consumed in order); sin0/cos0 land
            # between the wq halves so the k0 RoPE muls are never
            # table-gated.
            dma_w(wk_sb, wkT, slice(0, KO // 2))
            dma_x(0, slice(0, KO // 2))
            dma_w(wk_sb, wkT, slice(KO // 2, KO))
            dma_x(0, slice(KO // 2, KO))
            nc.sync.dma_start(sin_sb[:, :, 0:SC], sin2[:, :, 0:SC])
            nc.sync.dma_start(cos_sb[:, :, 0:SC], cos2[:, :, 0:SC])
            dma_w(wq_sb, wqT, slice(0, KO // 2))
            dma_w(wq_sb, wqT, slice(KO // 2, KO))
            dma_x(1)
            dma_cs(1)
            dma_x(2)
            dma_cs(2)
            dma_x(3)
            dma_w(wv_sb, wvT)
            dma_cs(3)
            for cs in range(2):
                nc.sync.dma_start(wo_sb[:, cs], woT[cs * 128:(cs + 1) * 128, :])

            # ---- projections + RoPE ----
            def qk_thunks(w_sb, dst_cs, c, chunked=False, defer_adds=False,
                          on_mm=False, nmm=4, hh1_dve=False):
                """Projection chunk as a list of thunks: matmul pieces (nmm
                each) + one RoPE piece, so the PE work can interleave
                between score groups without starving ACT. H set (half=1)
                first: the sin-term muls that read it overlap the L half's
                matmuls."""
                st = {}
                order = [(1, ko) for ko in range(KO)] + \
                        [(0, ko) for ko in range(KO)]

                def mm_piece(lo):
                    def f():
                        if "ps1" not in st:
                            # one PSUM tile PER HALF: tile-granular
                            # dependency tracking means the H-half RoPE
                            # muls would otherwise wait for the L half too
                            st["ps1"] = proj_pool.tile(
                                [128, SC], f32, tag="projH", name="qkpsh")
                            st["ps0"] = proj_pool.tile(
                                [128, SC], f32, tag="projL", name="qkpsl")
                        for half, ko in order[lo:lo + nmm]:
                            nc.tensor.matmul(
                                st[f"ps{half}"][:],
                                w_sb[:, ko, half * 128:(half + 1) * 128],
                                x_sb[c][:, ko, :],
                                start=(ko == 0), stop=(ko == KO - 1),
                            )
                    return f

                def rope_piece():
                    ps1, ps0 = st["ps1"], st["ps0"]
                    sl = slice(c * SC, (c + 1) * SC)
                    tmc = rope.tile([128, 2, SC], bf16, tag="tmc")
                    tms = rope.tile([128, 2, SC], bf16, tag="tms")
                    nc.vector.tensor_mul(
                        out=tms[:, 0], in0=ps1[:], in1=sin_sb[:, 0, sl])
                    nc.vector.tensor_mul(
                        out=tmc[:, 1], in0=ps1[:], in1=cos_sb[:, 1, sl])
                    nc.vector.tensor_mul(
                        out=tmc[:, 0], in0=ps0[:], in1=cos_sb[:, 0, sl])
                    nc.vector.tensor_mul(
                        out=tms[:, 1], in0=ps0[:], in1=sin_sb[:, 1, sl])
                    # add + partition reshuffle in one: out block (t, hh, d)
                    # of the [c, s] layout takes LH partitions 64t+32hh at
                    # pair d. hh=0 rows feed the even-h steps first, so they
                    # go on DVE (fast); hh=1 rows are needed a step later
                    # and go on Pool.
                    for hh in range(2):
                        # defer_adds (k0 only): all adds on Pool so DVE can
                        # start the next chunk's muls immediately; hh1_dve
                        # (k2/k3): hh1 adds on DVE, whose chain finishes
                        # before Pool's -- the hh1 chain bounds step 1
                        eng = nc.vector if (hh == 0 and not defer_adds) \
                            or (hh == 1 and hh1_dve) else nc.gpsimd
                        for t in range(2):
                            sp = 64 * t + 32 * hh
                            for d in range(2):
                                dp = 64 * hh + 32 * d
                                if chunked:
                                    dst = dst_cs[t][c][dp:dp + 32, :]
                                else:
                                    dst = dst_cs[t][dp:dp + 32, sl]
                                eng.tensor_add(
                                    out=dst,
                                    in0=tmc[sp:sp + 32, d, :],
                                    in1=tms[sp:sp + 32, d, :])

                return [mm_piece(lo) for lo in range(0, 2 * KO, nmm)] \
                    + [rope_piece]

            def qk_chunk(w_sb, dst_cs, c, **kw):
                for t in qk_thunks(w_sb, dst_cs, c, nmm=2 * KO, **kw):
                    t()

            def v_tile(st):
                ps = pop_pool.tile([128, CPC], f32, tag="bp", name="pv")
                for ko in range(KO):
                    nc.tensor.matmul(
                        ps[:],
                        x_sb[st // 4][:, ko, (st % 4) * 128:(st % 4 + 1) * 128],
                        wv_sb[:, ko, :],
                        start=(ko == 0), stop=(ko == KO - 1),
                    )
                psv = ps[:].rearrange("p (h e) -> p h e", e=HD)
                vt_v = v_bf[st][:].rearrange("p (h w) -> p h w", w=VW)
                nc.vector.tensor_copy(out=vt_v[:, :, 0:HD], in_=psv[:])

            # ---- attention steps ----
            def score_group(qc, h, g, p_bf):
                cs, pof = h // 2, (h % 2) * HD
                sps = mm_pool.tile([128, 2, SC], f32, tag="mm")
                for kti in range(2):
                    kt = g * 2 + kti
                    nc.tensor.matmul(
                        sps[:, kti],
                        k_cs[cs][kt // 4][pof:pof + HD,
                                          (kt % 4) * 128:(kt % 4 + 1) * 128],
                        q_cs[cs][pof:pof + HD, qc * SC:(qc + 1) * SC],
                        start=True, stop=True,
                    )
                nc.scalar.activation(
                    out=p_bf[:, g * 2:(g + 1) * 2], in_=sps[:],
                    func=AF.Exp, scale=0.125, bias=bias_sb[:],
                )

            def av_finish(qc, h, avb, av_q):
                # normalization stays on DVE: Pool/GPSIMD cannot read PSUM
                hh = h % 2
                avp = avb[:].rearrange("p (a b) -> p a b", b=128)
                rec = ptmp.tile([128, NSC], f32, tag="rec")
                nc.vector.reciprocal(out=rec[:], in_=avp[:, :, HD:HD + 1])
                for qt in range(4):
                    nc.vector.tensor_scalar_mul(
                        out=av_q[:, qt, hh], in0=avp[:, qt, 0:HD],
                        scalar1=rec[:, qt:qt + 1],
                    )

            def avt_transpose(qc, cs, av_q, tail=False):
                # [q, (hh d)] -> [c, q] via the SBUF crossbar (bf16 2-byte).
                # In the tail the ACT queue is idle, so alternate the two
                # HWDGE queues to halve the issue latency.
                for qt in range(4):
                    qo = qc * SC + qt * 128
                    eng = nc.scalar if (tail and qt % 2) else nc.sync
                    eng.dma_start(
                        avt_sb[:, cs, qo:qo + 128],
                        av_q[:, qt].rearrange("p a b -> p (a b)"),
                        transpose=True,
                    )

            def o_st(qc, sti, alt_q=False):
                st = qc * 4 + sti
                o_sb = pc.tile([128, 2, SC], bf16, tag="o_sb")
                for oc in range(2):
                    po = pop_pool.tile([128, SC], f32, tag="bp", name="po")
                    for cs in range(2):
                        nc.tensor.matmul(
                            po[:],
                            avt_sb[:, cs, st * 128:(st + 1) * 128],
                            wo_sb[:, cs, oc * SC:(oc + 1) * SC],
                            start=(cs == 0), stop=(cs == 1),
                        )
                    # Pool/GPSIMD cannot read PSUM; in the tail ACT is idle
                    # and can, so split the copies DVE/ACT there
                    if alt_q and oc == 1:
                        nc.scalar.copy(out=o_sb[:, oc], in_=po[:])
                    else:
                        nc.vector.tensor_copy(out=o_sb[:, oc], in_=po[:])
                # tail stores split per-oc on alternating queues so the
                # last store waits only the last copy; mid-kernel stores
                # stay whole on the software DGE (HWDGE is busy there)
                if alt_q:
                    for oc in range(2):
                        eng = nc.sync if (sti + oc) % 2 else nc.gpsimd
                        eng.dma_start(
                            out[st * 128:(st + 1) * 128,
                                oc * SC:(oc + 1) * SC],
                            o_sb[:, oc])
                else:
                    nc.gpsimd.dma_start(
                        out[st * 128:(st + 1) * 128, :],
                        o_sb[:].rearrange("p a s -> p (a s)"))

            # ---- ramp: k0, q0, k1 so the first score group can run ~15us
            # in while later k chunks interleave between early groups ----
            qk_chunk(wk_sb, k_cs, 0, chunked=True, on_mm=True)
            qk_chunk(wq_sb, q_cs, 0)
            qk_chunk(wk_sb, k_cs, 1, chunked=True, on_mm=True)

            # extras[(step, group)] -> thunks emitted right after that
            # score group's exp. Budgets keep per-step PE work under the
            # ACT step time (~8.3us): scores 3.4 + extras <= ~5us.
            extras = {
                (2, 1): [lambda: v_tile(5)],
                (2, 3): [lambda: v_tile(6)],
                (2, 5): [lambda: v_tile(7)],
                (2, 7): [lambda: v_tile(8), lambda: v_tile(9)],
                (3, 1): [lambda: v_tile(10)],
                (3, 3): [lambda: v_tile(11)],
                (4, 1): [lambda: v_tile(12)],
                (4, 3): [lambda: v_tile(13)],
                (4, 5): [lambda: v_tile(14)],
                (4, 7): [lambda: v_tile(15)],
            }
            # late q chunks interleave as 4-matmul pieces every other
            # group, so ACT never sees a contiguous 3.4us projection block
            for spots, qc_ in ((((3, 0), (3, 2), (3, 4), (3, 6)), 1),
                               (((5, 3), (6, 1), (6, 5), (7, 1)), 2),
                               (((9, 3), (10, 0), (10, 2), (10, 4)), 3)):
                # (placement tuned so no step exceeds the ACT budget)
                th = qk_thunks(wq_sb, q_cs, qc_)
                for pi in range(4):
                    extras.setdefault(spots[pi], []).append(th[pi])
                extras.setdefault(spots[3], []).append(th[4])
            # drains emitted at the end of each step (AV lags 4 steps, then
            # 2 drains/step from step 11 so only step 15's own drain is
            # left for the tail); o-projections split 2 s-tiles at a time
            # to level the per-step PE load.
            # drains spread so every step's PE load stays under the ~8.3us
            # ACT step time, and starting only at step 5 -- their AV pieces
            # read every v tile, so all v_tile emissions (through step 4)
            # must precede them. o-projections run at least one step after
            # the drain that wrote their avt slices.
            drain_at = {5: [0, 1], 6: [2, 3], 7: [4, 5], 8: [6],
                        9: [7, 8], 10: [9], 11: [10], 12: [11], 13: [12],
                        14: [13], 15: [14]}
            o_at = {8: (0, [0, 1, 2, 3]), 11: (1, [0, 1]),
                    12: (1, [2]), 13: (1, [3]), 14: (2, [0, 1]),
                    15: (2, [2, 3])}

            steps = [(qc, h) for qc in range(NSC) for h in range(HPC)]
            hist = {}
            av_q = [None]
            drain_avb = {}

            def drain_thunks(j):
                # one drain = 4 AV q-tile pieces (~0.43us PE each) + a
                # finish (rec+norm+transpose); spread across group slots so
                # ACT never waits behind a contiguous AV block
                def av_piece(qt):
                    def f():
                        (pqc, ph), pp = hist[j]
                        if j not in drain_avb:
                            drain_avb[j] = pop_pool.tile(
                                [128, SC], f32, tag="bp", name="avb")
                        avp = drain_avb[j][:].rearrange(
                            "p (a b) -> p a b", b=128)
                        for kt in range(NST):
                            nc.tensor.matmul(
                                avp[:, qt, 0:VW],
                                pp[:, kt, qt * 128:(qt + 1) * 128],
                                v_bf[kt][:, ph * VW:(ph + 1) * VW],
                                start=(kt == 0), stop=(kt == NST - 1),
                            )
                    return f

                def fin():
                    (pqc, ph), pp = hist.pop(j)
                    avb = drain_avb.pop(j)
                    if ph % 2 == 0:
                        av_q[0] = avq_pool.tile(
                            [128, NSC, 2, HD], bf16, tag="avq", name="av_q")
                    av_finish(pqc, ph, avb, av_q[0])
                    if ph % 2 == 1:
                        avt_transpose(pqc, ph // 2, av_q[0])

                return [av_piece(qt) for qt in range(4)] + [fin]

            # assemble the per-step work queue: group-keyed extras, then
            # drains (their avt feeds this step's o-projections) and o
            # s-tiles distributed over the group slots
            work_at = {}
            for (i, g), ths in extras.items():
                work_at.setdefault(i, {}).setdefault(g, []).extend(ths)
            for i in range(len(steps)):
                # per-step work order: o-projections first (their avt
                # inputs are at least a step old), then older drains, then
                # lag-1 drains (whose AV needs the previous step's LAST
                # exp, landing ~2 exp-slots into this step)
                pending = []
                if i in o_at:
                    oqc, stis = o_at[i]
                    pending.extend(
                        (lambda q=oqc, s=sti: o_st(q, s)) for sti in stis)
                for j in sorted(drain_at.get(i, []), key=lambda j: -(i - j)):
                    pending.extend(drain_thunks(j))
                n = len(pending)
                for k, th in enumerate(pending):
                    g = min(7, 2 + k * 5 // max(n, 1))
                    work_at.setdefault(i, {}).setdefault(g, []).append(th)

            # steps 0 and 1 interleave group-wise: step 1's exps depend only
            # on the Pool-side hh1 RoPE adds, which complete during step
            # 0's DVE rope-chain waits, so they fill step 0's ACT gaps.
            p01 = []
            for i in range(2):
                p_bf = pb.tile([128, NST, SC], bf16, tag="p_bf", name="pbf01")
                hist[i] = (steps[i], p_bf)
                p01.append(p_bf)
            seq01 = [
                ("s", 0, 0), ("k", 2), ("s", 0, 1), ("s", 1, 0), ("s", 1, 1),
                ("s", 0, 2), ("s", 0, 3),
                ("k", 3), ("v", 0),
                ("s", 1, 2), ("s", 1, 3), ("v", 1), ("s", 0, 4), ("s", 0, 5),
                ("v", 2), ("s", 1, 4), ("s", 1, 5), ("v", 3),
                ("s", 0, 6), ("s", 0, 7), ("v", 4), ("s", 1, 6), ("s", 1, 7),
            ]
            for item in seq01:
                if item[0] == "s":
                    _, si, g = item
                    score_group(0, si, g, p01[si])
                elif item[0] == "k":
                    qk_chunk(wk_sb, k_cs, item[1], chunked=True,
                             hh1_dve=(item[1] == 3))
                else:
                    v_tile(item[1])

            last = len(steps) - 1
            for i, (qc, h) in list(enumerate(steps))[2:]:
                p_bf = pb.tile([128, NST, SC], bf16, tag="p_bf")
                hist[i] = ((qc, h), p_bf)
                for g in range(NST // 2):
                    score_group(qc, h, g, p_bf)
                    for thunk in work_at.get(i, {}).get(g, []):
                        thunk()

            # tail: the last drain runs as a per-q-tile pipeline -- AV,
            # normalize, transpose, o-project, store -- so each q-tile's
            # store starts as soon as its own chain is done
            (_, _), pp = hist.pop(last)
            # two alternating PSUM banks: a start=True matmul re-zeroes its
            # whole 2KB bank region, so staying in one bank would WAR
            # against the previous q-tile's norm reads
            avbs = [pop_pool.tile([128, SC], f32, tag="bp", name=f"avbt{z}")
                    for z in range(2)]
            for qt in range(4):
                avp = avbs[qt % 2][:].rearrange(
                    "p (a b) -> p a b", b=128)[:, qt // 2 * 2]
                for kt in range(NST):
                    nc.tensor.matmul(
                        avp[:, 0:VW],
                        pp[:, kt, qt * 128:(qt + 1) * 128],
                        v_bf[kt][:, (HPC - 1) * VW:HPC * VW],
                        start=(kt == 0), stop=(kt == NST - 1),
                    )
                # norm + transpose overlap the next q-tile's AV matmuls.
                # Transposes run on the idle PE via the identity trick (one
                # per free mm-ring bank -- no zero-region WAR), with the
                # PSUM->SBUF copies split DVE/ACT; this avoids the 4-deep
                # serial HWDGE transpose chain.
                rec = ptmp.tile([128, 1], f32, tag="rec1", name="rec1")
                nc.vector.reciprocal(out=rec[:], in_=avp[:, HD:HD + 1])
                nc.vector.tensor_scalar_mul(
                    out=av_q[0][:, qt, 1], in0=avp[:, 0:HD],
                    scalar1=rec[:],
                )
                trt = mm_pool.tile([128, 128], bf16, tag="mm", name="trt")
                nc.tensor.transpose(
                    trt[:], av_q[0][:, qt].rearrange("p a b -> p (a b)"),
                    ident_sb[:])
                qo = (NSC - 1) * SC + qt * 128
                if qt % 2:
                    nc.scalar.copy(out=avt_sb[:, 1, qo:qo + 128], in_=trt[:])
                else:
                    nc.vector.tensor_copy(
                        out=avt_sb[:, 1, qo:qo + 128], in_=trt[:])
            for qt in range(4):
                o_st(NSC - 1, qt, alt_q=True)

    _split_waits(nc, mybir)
    return nc


def _rope_tables():
    import ml_dtypes
    inv_freq = 1.0 / (BASE ** (np.arange(0, HD, 2, dtype=np.float32) / HD))
    t = np.arange(S, dtype=np.float32)
    freqs = np.einsum("i,j->ij", t, inv_freq)        # [S, 32]
    cos_t = np.cos(freqs).T.astype(np.float32)       # [32, S]
    sin_t = np.sin(freqs).T.astype(np.float32)
    rows = np.arange(128) % 32
    cos2 = np.stack([cos_t[rows], cos_t[rows]], axis=1)      # [128, 2, S]
    sin2 = np.stack([-sin_t[rows], sin_t[rows]], axis=1)
    return (cos2.astype(ml_dtypes.bfloat16), sin2.astype(ml_dtypes.bfloat16))


def _run(inputs, trace=False):
    global _cached
    import ml_dtypes
    from concourse.bass_utils import run_bass_kernel_spmd

    x = np.asarray(inputs["x"], dtype=np.float32)
    wq = np.asarray(inputs["wq"], dtype=np.float32)
    wk = np.asarray(inputs["wk"], dtype=np.float32)
    wv = np.asarray(inputs["wv"], dtype=np.float32)
    wo = np.asarray(inputs["wo"], dtype=np.float32)
    bq = np.asarray(inputs["bq"], dtype=np.float32)
    bk = np.asarray(inputs["bk"], dtype=np.float32)
    bv = np.asarray(inputs["bv"], dtype=np.float32)
    bo = np.asarray(inputs["bo"], dtype=np.float32)
    assert not (bq.any() or bk.any() or bv.any()), \
        "nonzero qkv biases not supported by this kernel build"

    if _cached is None:
        _cached = _build()
    nc = _cached

    cos2, sin2 = _rope_tables()
    # L/H channel order: position p -> head p//32, channel p%32 (+32 for H)
    p = np.arange(128)
    lorder = (p // 32) * HD + (p % 32)
    order = np.concatenate([lorder, lorder + 32])
    bf = ml_dtypes.bfloat16
    in_maps = []
    for core in range(N_CORES):
        b, g = divmod(core, GROUPS)
        cs = slice(g * CPC, (g + 1) * CPC)
        in_maps.append({
            "xT": np.ascontiguousarray(x[b].T).astype(bf),
            "wqT": np.ascontiguousarray(wq[cs][order].T).astype(bf),
            "wkT": np.ascontiguousarray(wk[cs][order].T).astype(bf),
            "wvT": np.ascontiguousarray(wv[cs].T).astype(bf),
            "woT": np.ascontiguousarray(wo[:, cs].T).astype(bf),
            "cos2": cos2,
            "sin2": sin2,
        })

    res = run_bass_kernel_spmd(
        nc, in_maps, core_ids=list(range(N_CORES)), trace=trace)

    outp = np.zeros((B, S, HID), dtype=np.float32)
    for core in range(N_CORES):
        b = core // GROUPS
        outp[b] += res.results[core]["out"].astype(np.float32)
    outp += bo
    return outp, res


def kernel(**inputs):
    outp, _ = _run(inputs, trace=False)
    return outp
